# revision 12
# baseline (speedup 1.0000x reference)
"""Mode-adaptive linear (MoE soft routing) Trainium2 kernel.

out[b, o] = sum_c weights[b, c] * (inputs[b, :] @ w[c])[o] + (weights @ bias)[b, o]

Strategy: data-parallel shard of the batch across 8 NeuronCores (1024 rows
each); w/bias replicated.  Per core, routing weights are folded into the
transposed input tiles (xs = wb * X^T, bf16), and each 128-row batch tile
owns one PSUM bank that accumulates the bias matmul plus all 8 experts'
K-chunk matmuls (K = 8*512 + 8) before a single eviction.

Engine/ring budget per core:
  PE     : 264 N=512 bf16 matmuls (~31us) + 32 bf16 transposes + 16 wb
           broadcast matmuls + 8 wt transposes + HAM warmup
  DVE    : xs routing-weight scaling in 2x mode (~18us)
  Scalar : x f32->bf16 casts + PSUM evictions (~18us)
  gpsimd ring : the 8MB expert tensor, cast f32->bf16 in flight (w_c
           arrives ~2.9*(c+1) us -> expert pacing groups {1,2,2,3})
  scalar ring : x tiles + identities;  sync ring: wt/b/e_pad + output
"""

import json
import types

import numpy as np

import concourse.bass as bass
import concourse.mybir as mybir
import concourse.tile as tile
from concourse.bass import ts
from concourse.bass_utils import run_bass_kernel_spmd

N_CORES = 8
B, D_IN, D_OUT, N_CTRL = 8192, 512, 512, 8
B_SHARD = B // N_CORES          # 1024 rows per core
P = 128
N_TILES = B_SHARD // P          # 8 batch tiles per core
KS = D_IN // P                  # 4 K-chunks of 128
F32 = mybir.dt.float32
BF16 = mybir.dt.bfloat16

# Expert pacing groups: ordered so group g's experts are already loaded by
# the time its matmuls start (w_c lands ~2.9*(c+1) us on the gpsimd ring).
PACING = [[0], [1, 2], [3, 4], [5, 6, 7]]
N_WARM = 4


def _consts(nc: bass.Bass, const_pool):
    """One-time constants, embedded in the NEFF and DMA'd to SBUF: identity
    matrices for PE transposes and the expert-selection matrix
    e_pad[p, c, m] = 1 iff p == c, so matmul(lhsT=e_pad[:, c], rhs=wt_pad)
    broadcasts wt_pad row c to all 128 output partitions."""
    import ml_dtypes

    identity_d = nc.inline_tensor(np.eye(P, dtype=np.float32), name="identity_const")
    identity = const_pool.tile([P, P], F32)
    nc.scalar.dma_start(identity, identity_d.ap())

    identity_bf_d = nc.inline_tensor(
        np.eye(P, dtype=ml_dtypes.bfloat16), name="identity_bf_const"
    )
    identity_bf = const_pool.tile([P, P], BF16)
    nc.scalar.dma_start(identity_bf, identity_bf_d.ap())

    e_np = np.zeros((P, N_CTRL, P), dtype=ml_dtypes.bfloat16)
    for c in range(N_CTRL):
        e_np[c, c, :] = 1.0
    e_d = nc.inline_tensor(e_np, name="e_pad_const")
    e_pad = const_pool.tile([P, N_CTRL, P], BF16)
    return identity, identity_bf, (e_pad, e_d)


def _body(nc: bass.Bass, tc: tile.TileContext, x_d, wt_d, w_d, b_d, o_d,
          identity, identity_bf, e_pad_pair, gate=None):
    e_pad, e_d = e_pad_pair
    with (
        tc.tile_pool(name="const", bufs=1) as const_pool,
        tc.tile_pool(name="xpool", bufs=4) as xpool,
        tc.tile_pool(name="xbfpool", bufs=3) as xbfpool,
        tc.tile_pool(name="xtpool", bufs=N_TILES) as xtpool,
        tc.tile_pool(name="xspool", bufs=2) as xspool,
        tc.tile_pool(name="opool", bufs=3) as opool,
        tc.tile_pool(name="ps", bufs=8, space="PSUM") as ps_pool,
    ):
        if gate is not None:
            # Serial-timing mode: the first DMA on each ring reads the
            # previous rep's final output tile (RAW dep) and writes back
            # values o_d already holds, serializing rep boundaries.
            nc.gpsimd.dma_start(o_d[0:1, 0:2], gate[0:1, 0:2])
            nc.scalar.dma_start(o_d[0:1, 2:4], gate[0:1, 2:4])
            nc.sync.dma_start(o_d[0:1, 4:6], gate[0:1, 4:6])

        # --- DMA issue order defines each ring's FIFO. ---
        # gpsimd SWDGE ring: the expert tensor only, cast f32->bf16 in
        # flight, one DMA per expert.  w_sb[p, c, k, o] = w[c, 128k+p, o].
        w_sb = const_pool.tile([P, N_CTRL, KS, D_OUT], BF16)
        for c in range(N_CTRL):
            nc.gpsimd.dma_start(
                w_sb[:, c], w_d[c].rearrange("(k p) o -> p k o", p=P)
            )

        # scalar HWDGE ring: x tiles (f32; cast on ScalarE later).
        x_f32s = []
        for t in range(N_TILES):
            x_f32 = xpool.tile([P, D_IN], F32, tag="x_f32")
            nc.scalar.dma_start(x_f32, x_d[ts(t, P), :])
            x_f32s.append(x_f32)

        # sync ring: routing weights, bias, selection matrix.
        wt_nat = const_pool.tile([P, N_TILES, N_CTRL], F32)
        nc.sync.dma_start(wt_nat, wt_d.rearrange("(t p) c -> p t c", p=P))
        b_f32 = const_pool.tile([N_CTRL, D_OUT], F32)
        nc.sync.dma_start(b_f32, b_d)
        nc.sync.dma_start(e_pad, e_d.ap())

        # --- HAM warmup: keep PE busy from ~0.4us so the clock-gate lifts
        # early.  First allocation sizes the shared PSUM slots to a bank.
        for _ in range(N_WARM):
            warm_ps = ps_pool.tile([P, D_OUT], F32, tag="ps", name="warm_ps")
            nc.tensor.matmul(
                warm_ps[:, 0:P], lhsT=identity_bf, rhs=identity_bf,
                start=True, stop=True,
            )

        # --- wt^T via PE transpose, zero-padded to 128 partitions (bf16).
        wt_pad = const_pool.tile([P, B_SHARD], BF16)
        nc.vector.memset(wt_pad, 0.0)
        for t in range(N_TILES):
            wtt_ps = ps_pool.tile([N_CTRL, P], F32, tag="ps")
            nc.tensor.transpose(wtt_ps, wt_nat[:, t, :], identity)
            nc.scalar.copy(wt_pad[0:N_CTRL, ts(t, P)], wtt_ps)

        # Bias, zero-padded the same way.
        b_pad = const_pool.tile([P, D_OUT], BF16)
        nc.vector.memset(b_pad, 0.0)
        nc.vector.tensor_copy(b_pad[0:N_CTRL, :], b_f32)

        # --- X^T per tile: ScalarE cast to bf16, PE transpose (1 cyc/row),
        # ScalarE eviction.
        xts = []

        def transpose_tile(t):
            x_bf = xbfpool.tile([P, D_IN], BF16, tag="x_bf")
            nc.scalar.copy(x_bf, x_f32s[t])
            tr_ps = ps_pool.tile([P, KS, P], BF16, tag="ps")
            xt = xtpool.tile([P, KS, P], BF16)
            for k in range(KS):
                nc.tensor.transpose(tr_ps[:, k, :], x_bf[:, ts(k, P)],
                                    identity_bf)
            nc.scalar.copy(xt, tr_ps)
            xts.append(xt)

        transpose_tile(0)
        transpose_tile(1)

        # --- wb[p, c, b] = weights[b, c] on every partition, via selection
        # matmuls (these double as extra HAM warmup); remaining x tiles'
        # transposes ride between experts.
        wb = const_pool.tile([P, N_CTRL, B_SHARD], BF16)
        for c in range(N_CTRL):
            for h in range(B_SHARD // 512):
                bc_ps = ps_pool.tile([P, 512], F32, tag="ps")
                nc.tensor.matmul(
                    bc_ps, lhsT=e_pad[:, c, :], rhs=wt_pad[:, ts(h, 512)],
                    start=True, stop=True,
                )
                nc.scalar.copy(wb[:, c, ts(h, 512)], bc_ps)
            if c < 3:
                transpose_tile(2 * c + 2)
                transpose_tile(2 * c + 3)

        # --- Main accumulation: one PSUM bank per batch tile; bias first,
        # then all experts' K-chunks in pacing groups; single eviction.
        accs = [None] * N_TILES
        o_sb = None
        for gi, grp in enumerate(PACING):
            c0 = grp[0]
            cpg = len(grp)
            last_g = gi == len(PACING) - 1
            for t in range(N_TILES):
                # xs[:, k, ci, :] = X^T * wb — layout [k, ci, b] keeps every
                # operand's innermost stride 1 so the DVE runs in 2x mode.
                xs = xspool.tile([P, KS, cpg, P], BF16, tag=f"xs{gi}", bufs=2)
                nc.vector.tensor_mul(
                    xs,
                    xts[t][:, :, None, :].to_broadcast([P, KS, cpg, P]),
                    wb[:, None, c0:c0 + cpg, ts(t, P)].to_broadcast(
                        [P, KS, cpg, P]
                    ),
                )
                if gi == 0:
                    accs[t] = ps_pool.tile([P, D_OUT], F32, tag="ps")
                    nc.tensor.matmul(
                        accs[t], lhsT=wt_pad[:, ts(t, P)], rhs=b_pad,
                        start=True, stop=False,
                    )
                for ci in range(cpg):
                    c = c0 + ci
                    for k in range(KS):
                        nc.tensor.matmul(
                            accs[t],
                            lhsT=xs[:, k, ci, :],
                            rhs=w_sb[:, c, k, :],
                            start=False,
                            stop=(last_g and ci == cpg - 1 and k == KS - 1),
                        )
                if last_g:
                    o_sb = opool.tile([P, D_OUT], F32, tag="o_sb")
                    nc.scalar.copy(o_sb, accs[t])
                    nc.sync.dma_start(o_d[ts(t, P), :], o_sb)
        return o_sb


def _split_multi_waits(bir: dict) -> dict:
    """The walrus build in this container supports at most ONE sync-wait per
    instruction ("Too many sync wait commands" at codegen otherwise).  Tile's
    scheduler freely attaches several.  Split: keep the last wait on the
    instruction and hoist the others onto standalone same-engine
    EventSemaphore instructions inserted immediately before it — identical
    semantics (the engine blocks at the same program point)."""
    ctr = 0
    for func in bir["functions"]:
        for bb in func["blocks"]:
            new_insts = []
            for inst in bb["instructions"]:
                si = inst.get("sync_info")
                waits = si.get("on_wait") if si else None
                if waits and len(waits) > 1:
                    for w in waits[:-1]:
                        ctr += 1
                        new_insts.append(
                            {
                                "debug": inst.get("debug", 0),
                                "engine": inst["engine"],
                                "ins": [],
                                "outs": [],
                                "name": f"{inst['name']}-wsplit{ctr}",
                                "opcode": "EventSemaphore",
                                "sync_info": {"on_update": [], "on_wait": [w]},
                            }
                        )
                    si["on_wait"] = [waits[-1]]
                new_insts.append(inst)
            bb["instructions"] = new_insts
    return bir


_ORIG_TO_JSON_BYTES = bass.Bass.to_json_bytes


def _patched_to_json_bytes(self) -> bytes:
    bir = json.loads(_ORIG_TO_JSON_BYTES(self))
    _split_multi_waits(bir)
    return json.dumps(bir).encode()


_NC_CACHE = {}


def _build(reps: int = 1, serial: bool = False) -> bass.Bass:
    key = (reps, serial)
    if key in _NC_CACHE:
        return _NC_CACHE[key]
    nc = bass.Bass(
        "TRN2",
        target_bir_lowering=False,
        debug=False,
        enable_asserts=False,
        num_devices=N_CORES,
    )
    x_d = nc.dram_tensor("x_in", [B_SHARD, D_IN], F32, kind="ExternalInput").ap()
    wt_d = nc.dram_tensor("wt_in", [B_SHARD, N_CTRL], F32, kind="ExternalInput").ap()
    w_d = nc.dram_tensor("w_in", [N_CTRL, D_IN, D_OUT], F32, kind="ExternalInput").ap()
    b_d = nc.dram_tensor("b_in", [N_CTRL, D_OUT], F32, kind="ExternalInput").ap()
    o_d = nc.dram_tensor("out", [B_SHARD, D_OUT], F32, kind="ExternalOutput").ap()
    with tile.TileContext(nc) as tc:
        with tc.tile_pool(name="global_const", bufs=1) as gconst:
            identity, identity_bf, e_pad = _consts(nc, gconst)
            gate = None
            for _ in range(reps):
                out_tile = _body(
                    nc, tc, x_d, wt_d, w_d, b_d, o_d, identity, identity_bf,
                    e_pad, gate=gate,
                )
                if serial:
                    gate = out_tile
    nc.to_json_bytes = types.MethodType(_patched_to_json_bytes, nc)
    _NC_CACHE[key] = nc
    return nc


def kernel(inputs, weights, w, b, _trace=False, _reps=1, _serial=False):
    nc = _build(_reps, _serial)
    inputs = np.ascontiguousarray(inputs, dtype=np.float32)
    weights = np.ascontiguousarray(weights, dtype=np.float32)
    w = np.ascontiguousarray(w, dtype=np.float32)
    b = np.ascontiguousarray(b, dtype=np.float32)

    in_maps = []
    for i in range(N_CORES):
        sl = slice(i * B_SHARD, (i + 1) * B_SHARD)
        in_maps.append(
            {
                "x_in": inputs[sl],
                "wt_in": weights[sl],
                "w_in": w,
                "b_in": b,
            }
        )
    res = run_bass_kernel_spmd(
        nc, in_maps, core_ids=list(range(N_CORES)), trace=_trace
    )
    out = np.concatenate([r["out"] for r in res.results], axis=0)
    if _trace:
        return out, res
    return out


# revision 27
# speedup vs baseline: 1.0524x; 1.0524x over previous
"""Mode-adaptive linear (MoE soft routing) Trainium2 kernel.

out[b, o] = sum_c weights[b, c] * (inputs[b, :] @ w[c])[o] + (weights @ bias)[b, o]

Strategy: data-parallel shard of the batch across 8 NeuronCores (1024 rows
each); w/bias replicated.  Per core, routing weights are folded into the
transposed input tiles (xs = wb * X^T, bf16), and each 128-row batch tile
owns one PSUM bank that accumulates the bias matmul plus all 8 experts'
K-chunk matmuls (K = 8*512 + 8) before a single eviction.

Engine/ring budget per core:
  PE     : 264 N=512 bf16 matmuls (~31us) + 32 bf16 transposes + 16 wb
           broadcast matmuls + 8 wt transposes + HAM warmup
  DVE    : xs routing-weight scaling in 2x mode (~18us)
  Scalar : x f32->bf16 casts + PSUM evictions (~18us)
  gpsimd ring : the 8MB expert tensor, cast f32->bf16 in flight (w_c
           arrives ~2.9*(c+1) us -> expert pacing groups {1,2,2,3})
  scalar ring : x tiles + identities;  sync ring: wt/b/e_pad + output
"""

import json
import types

import numpy as np

import concourse.bass as bass
import concourse.mybir as mybir
import concourse.tile as tile
from concourse.bass import ts
from concourse.bass_utils import run_bass_kernel_spmd

N_CORES = 8
B, D_IN, D_OUT, N_CTRL = 8192, 512, 512, 8
B_SHARD = B // N_CORES          # 1024 rows per core
P = 128
N_TILES = B_SHARD // P          # 8 batch tiles per core
KS = D_IN // P                  # 4 K-chunks of 128
F32 = mybir.dt.float32
BF16 = mybir.dt.bfloat16

# Expert pacing groups: ordered so group g's experts are already loaded by
# the time its matmuls start (w_c lands ~2.9*(c+1) us on the gpsimd ring).
PACING = [[0], [1, 2], [3, 4], [5, 6, 7]]
N_WARM = 4


def _consts(nc: bass.Bass, const_pool):
    """One-time constants, embedded in the NEFF and DMA'd to SBUF: identity
    matrices for PE transposes.  identity_bf[:, c] doubles as the expert-
    selection vector: matmul(lhsT=identity_bf[:, c:c+1] broadcast to 128
    columns, rhs) replicates rhs row c onto all 128 output partitions."""
    import ml_dtypes

    identity_bf_d = nc.inline_tensor(
        np.eye(P, dtype=ml_dtypes.bfloat16), name="identity_bf_const"
    )
    identity_bf = const_pool.tile([P, P], BF16)
    nc.sync.dma_start(identity_bf, identity_bf_d.ap())

    identity_d = nc.inline_tensor(np.eye(P, dtype=np.float32), name="identity_const")
    identity = const_pool.tile([P, P], F32)
    nc.sync.dma_start(identity, identity_d.ap())
    return identity, identity_bf


def _body(nc: bass.Bass, tc: tile.TileContext, x_d, wt_d, w_d, b_d, o_d,
          identity, identity_bf, gate=None):
    with (
        tc.tile_pool(name="const", bufs=1) as const_pool,
        tc.tile_pool(name="xpool", bufs=4) as xpool,
        tc.tile_pool(name="xbfpool", bufs=3) as xbfpool,
        tc.tile_pool(name="xtpool", bufs=N_TILES) as xtpool,
        tc.tile_pool(name="xspool", bufs=2) as xspool,
        tc.tile_pool(name="opool", bufs=3) as opool,
        tc.tile_pool(name="ps", bufs=8, space="PSUM") as ps_pool,
    ):
        if gate is not None:
            # Serial-timing mode: the first DMA on each ring reads the
            # previous rep's final output tile (RAW dep) and writes back
            # values o_d already holds, serializing rep boundaries.
            nc.gpsimd.dma_start(o_d[0:1, 0:2], gate[0:1, 0:2])
            nc.scalar.dma_start(o_d[0:1, 2:4], gate[0:1, 2:4])
            nc.sync.dma_start(o_d[0:1, 4:6], gate[0:1, 4:6])

        # --- DMA issue order defines each ring's FIFO. ---
        # gpsimd SWDGE ring: the expert tensor only, cast f32->bf16 in
        # flight, one DMA per expert (each dma_start costs ~1us of SWDGE
        # descriptor generation, so fewer is better).
        # w_sb[p, c, k, o] = w[c, 128k+p, o].
        w_sb = const_pool.tile([P, N_CTRL, KS, D_OUT], BF16)
        for c in range(N_CTRL):
            nc.gpsimd.dma_start(
                w_sb[:, c], w_d[c].rearrange("(k p) o -> p k o", p=P)
            )

        # x tiles (f32; cast on ScalarE) in 3 batches — each dma_start holds
        # the issuing sequencer + HWDGE, so batch.  The first two batches go
        # on the sync ring (SP has nothing else to issue early, and the
        # scalar sequencer must stay free for the first casts); the tail
        # batch goes on the scalar ring.
        x_batches = []          # (first_t, ntiles, tile)
        for eng, t0b, nt in ((nc.scalar, 0, 2), (nc.scalar, 2, 2),
                             (nc.scalar, 4, 4)):
            x_f32 = xpool.tile([P, nt, D_IN], F32, tag=f"x_f32_{t0b}",
                               name=f"x_f32_{t0b}", bufs=2)
            eng.dma_start(
                x_f32,
                x_d[t0b * P:(t0b + nt) * P, :].rearrange(
                    "(t p) i -> p t i", p=P
                ),
            )
            x_batches.append((t0b, nt, x_f32))

        def x_f32_slice(t):
            for t0b, nt, x_f32 in x_batches:
                if t0b <= t < t0b + nt:
                    return x_f32[:, t - t0b, :]
            raise AssertionError

        # sync ring (behind the x batches): routing weights + bias (tiny).
        wt_nat = const_pool.tile([P, N_TILES, N_CTRL], F32)
        nc.sync.dma_start(wt_nat, wt_d.rearrange("(t p) c -> p t c", p=P))
        b_f32 = const_pool.tile([N_CTRL, D_OUT], F32)
        nc.sync.dma_start(b_f32, b_d)

        # --- ScalarE runs strictly in program order: the first two x casts
        # must lead so the first PE transposes aren't head-of-line blocked.
        x_bfs = {}
        for t in range(2):
            x_bf = xbfpool.tile([P, D_IN], BF16, tag="x_bf")
            nc.scalar.copy(x_bf, x_f32_slice(t))
            x_bfs[t] = x_bf

        # --- HAM warmup: N=512 dummy matmuls (broadcast rhs) keep PE busy
        # from ~0.4us so the clock-gate lifts early.  First allocation also
        # sizes the shared PSUM slots to a full bank.
        warm_rhs = identity_bf[:, None, :].to_broadcast([P, KS, P])
        for _ in range(N_WARM):
            warm_ps = ps_pool.tile([P, D_OUT], F32, tag="ps", name="warm_ps")
            nc.tensor.matmul(
                warm_ps, lhsT=identity_bf, rhs=warm_rhs,
                start=True, stop=True,
            )

        # --- X^T per tile: ScalarE cast to bf16, PE transpose (1 cyc/row),
        # ScalarE eviction.
        xts = []

        def transpose_tile(t):
            if t in x_bfs:
                x_bf = x_bfs[t]
            else:
                x_bf = xbfpool.tile([P, D_IN], BF16, tag="x_bf")
                nc.scalar.copy(x_bf, x_f32_slice(t))
            tr_ps = ps_pool.tile([P, KS, P], BF16, tag="ps")
            xt = xtpool.tile([P, KS, P], BF16)
            for k in range(KS):
                nc.tensor.transpose(tr_ps[:, k, :], x_bf[:, ts(k, P)],
                                    identity_bf)
            nc.scalar.copy(xt, tr_ps)
            xts.append(xt)

        transpose_tile(0)
        transpose_tile(1)

        # --- wt^T via PE transpose, zero-padded to 128 partitions (bf16);
        # wb[p, c, b] = weights[b, c] on every partition, via selection
        # matmuls: identity_bf[:, c] broadcast over the 128 lhsT columns
        # replicates wt_pad row c onto all output partitions.
        wt_pad = const_pool.tile([P, B_SHARD], BF16)
        nc.vector.memset(wt_pad, 0.0)
        wb = const_pool.tile([P, N_CTRL, B_SHARD], BF16)

        def wtt_half(h):
            for t in range(4 * h, 4 * h + 4):
                wtt_ps = ps_pool.tile([N_CTRL, P], F32, tag="ps")
                nc.tensor.transpose(wtt_ps, wt_nat[:, t, :], identity)
                nc.scalar.copy(wt_pad[0:N_CTRL, ts(t, P)], wtt_ps)

        def bc_expert(c, half=None):
            sel = identity_bf[:, c:c + 1].to_broadcast([P, P])
            for h in range(2) if half is None else (half,):
                bc_ps = ps_pool.tile([P, 512], F32, tag="ps")
                nc.tensor.matmul(
                    bc_ps, lhsT=sel, rhs=wt_pad[:, ts(h, 512)],
                    start=True, stop=True,
                )
                nc.scalar.copy(wb[:, c, ts(h, 512)], bc_ps)

        # First half of wb[c0] becomes available before the second half of
        # the wt transposes, so the first xs ops aren't gated on all of it.
        wtt_half(0)
        bc_expert(0, half=0)
        wtt_half(1)
        bc_expert(0, half=1)

        # Bias, zero-padded the same way.
        b_pad = const_pool.tile([P, D_OUT], BF16)
        nc.vector.memset(b_pad, 0.0)
        nc.vector.tensor_copy(b_pad[0:N_CTRL, :], b_f32)

        # --- Main accumulation: one PSUM bank per batch tile; bias first,
        # then all experts' K-chunks in pacing groups; single eviction.
        # Transposes of later x tiles and the remaining wb experts ride
        # inside the group-0 loop (PE would otherwise idle while group 0
        # is paced by the x stream).
        accs = [None] * N_TILES
        o_sb = None
        for gi, grp in enumerate(PACING):
            c0 = grp[0]
            cpg = len(grp)
            last_g = gi == len(PACING) - 1
            for t in range(N_TILES):
                if gi == 0 and t >= 2:
                    transpose_tile(t)
                if gi == 0 and 1 <= t < N_CTRL:
                    bc_expert(t)
                # xs[:, k, ci, :] = X^T * wb — layout [k, ci, b] keeps every
                # operand's innermost stride 1 so the DVE runs in 2x mode.
                xs = xspool.tile([P, KS, cpg, P], BF16, tag=f"xs{gi}", bufs=2)
                nc.vector.tensor_mul(
                    xs,
                    xts[t][:, :, None, :].to_broadcast([P, KS, cpg, P]),
                    wb[:, None, c0:c0 + cpg, ts(t, P)].to_broadcast(
                        [P, KS, cpg, P]
                    ),
                )
                if gi == 0:
                    accs[t] = ps_pool.tile([P, D_OUT], F32, tag="ps",
                                           name=f"acc{t}")
                    nc.tensor.matmul(
                        accs[t], lhsT=wt_pad[:, ts(t, P)], rhs=b_pad,
                        start=True, stop=False,
                    )
                for ci in range(cpg):
                    c = c0 + ci
                    for k in range(KS):
                        nc.tensor.matmul(
                            accs[t],
                            lhsT=xs[:, k, ci, :],
                            rhs=w_sb[:, c, k, :],
                            start=False,
                            stop=(last_g and ci == cpg - 1 and k == KS - 1),
                        )
                if last_g:
                    o_sb = opool.tile([P, D_OUT], F32, tag="o_sb")
                    if t == N_TILES - 1:
                        # Final tile: split eviction + store into halves so
                        # the out-DMA of the first half overlaps the second
                        # half's eviction (shrinks the kernel tail).
                        for h in range(2):
                            nc.scalar.copy(o_sb[:, ts(h, D_OUT // 2)],
                                           accs[t][:, ts(h, D_OUT // 2)])
                            nc.sync.dma_start(
                                o_d[ts(t, P), ts(h, D_OUT // 2)],
                                o_sb[:, ts(h, D_OUT // 2)],
                            )
                    else:
                        nc.scalar.copy(o_sb, accs[t])
                        nc.sync.dma_start(o_d[ts(t, P), :], o_sb)
        return o_sb


def _split_multi_waits(bir: dict) -> dict:
    """The walrus build in this container supports at most ONE sync-wait per
    instruction ("Too many sync wait commands" at codegen otherwise).  Tile's
    scheduler freely attaches several.  Split: keep the last wait on the
    instruction and hoist the others onto standalone same-engine
    EventSemaphore instructions inserted immediately before it — identical
    semantics (the engine blocks at the same program point)."""
    ctr = 0
    for func in bir["functions"]:
        for bb in func["blocks"]:
            new_insts = []
            for inst in bb["instructions"]:
                si = inst.get("sync_info")
                waits = si.get("on_wait") if si else None
                if waits and len(waits) > 1:
                    for w in waits[:-1]:
                        ctr += 1
                        new_insts.append(
                            {
                                "debug": inst.get("debug", 0),
                                "engine": inst["engine"],
                                "ins": [],
                                "outs": [],
                                "name": f"{inst['name']}-wsplit{ctr}",
                                "opcode": "EventSemaphore",
                                "sync_info": {"on_update": [], "on_wait": [w]},
                            }
                        )
                    si["on_wait"] = [waits[-1]]
                new_insts.append(inst)
            bb["instructions"] = new_insts
    return bir


_ORIG_TO_JSON_BYTES = bass.Bass.to_json_bytes


def _patched_to_json_bytes(self) -> bytes:
    bir = json.loads(_ORIG_TO_JSON_BYTES(self))
    _split_multi_waits(bir)
    return json.dumps(bir).encode()


_NC_CACHE = {}


def _build(reps: int = 1, serial: bool = False) -> bass.Bass:
    key = (reps, serial)
    if key in _NC_CACHE:
        return _NC_CACHE[key]
    nc = bass.Bass(
        "TRN2",
        target_bir_lowering=False,
        debug=False,
        enable_asserts=False,
        num_devices=N_CORES,
    )
    x_d = nc.dram_tensor("x_in", [B_SHARD, D_IN], F32, kind="ExternalInput").ap()
    wt_d = nc.dram_tensor("wt_in", [B_SHARD, N_CTRL], F32, kind="ExternalInput").ap()
    w_d = nc.dram_tensor("w_in", [N_CTRL, D_IN, D_OUT], F32, kind="ExternalInput").ap()
    b_d = nc.dram_tensor("b_in", [N_CTRL, D_OUT], F32, kind="ExternalInput").ap()
    o_d = nc.dram_tensor("out", [B_SHARD, D_OUT], F32, kind="ExternalOutput").ap()
    with tile.TileContext(nc) as tc:
        with tc.tile_pool(name="global_const", bufs=1) as gconst:
            identity, identity_bf = _consts(nc, gconst)
            gate = None
            for _ in range(reps):
                out_tile = _body(
                    nc, tc, x_d, wt_d, w_d, b_d, o_d, identity, identity_bf,
                    gate=gate,
                )
                if serial:
                    gate = out_tile
    nc.to_json_bytes = types.MethodType(_patched_to_json_bytes, nc)
    _NC_CACHE[key] = nc
    return nc


def kernel(inputs, weights, w, b, _trace=False, _reps=1, _serial=False):
    nc = _build(_reps, _serial)
    inputs = np.ascontiguousarray(inputs, dtype=np.float32)
    weights = np.ascontiguousarray(weights, dtype=np.float32)
    w = np.ascontiguousarray(w, dtype=np.float32)
    b = np.ascontiguousarray(b, dtype=np.float32)

    in_maps = []
    for i in range(N_CORES):
        sl = slice(i * B_SHARD, (i + 1) * B_SHARD)
        in_maps.append(
            {
                "x_in": inputs[sl],
                "wt_in": weights[sl],
                "w_in": w,
                "b_in": b,
            }
        )
    res = run_bass_kernel_spmd(
        nc, in_maps, core_ids=list(range(N_CORES)), trace=_trace
    )
    out = np.concatenate([r["out"] for r in res.results], axis=0)
    if _trace:
        return out, res
    return out


# revision 28
# speedup vs baseline: 1.2314x; 1.1701x over previous
"""Mode-adaptive linear (MoE soft routing) Trainium2 kernel.

out[b, o] = sum_c weights[b, c] * (inputs[b, :] @ w[c])[o] + (weights @ bias)[b, o]

Strategy: data-parallel shard of the batch across 8 NeuronCores (1024 rows
each); w/bias replicated.  Per core, routing weights are folded into the
transposed input tiles (xs = wb * X^T, bf16), and each 128-row batch tile
owns one PSUM bank that accumulates the bias matmul plus all 8 experts'
K-chunk matmuls (K = 8*512 + 8) before a single eviction.

Engine/ring budget per core:
  PE     : 264 N=512 bf16 matmuls (~31us) + 32 bf16 transposes + 16 wb
           broadcast matmuls + 8 wt transposes + HAM warmup
  DVE    : xs routing-weight scaling in 2x mode (~18us)
  Scalar : x f32->bf16 casts + PSUM evictions (~18us)
  gpsimd ring : the 8MB expert tensor, cast f32->bf16 in flight (w_c
           arrives ~2.9*(c+1) us -> expert pacing groups {1,2,2,3})
  scalar ring : x tiles + identities;  sync ring: wt/b/e_pad + output
"""

import json
import types

import numpy as np

import concourse.bass as bass
import concourse.mybir as mybir
import concourse.tile as tile
from concourse.bass import ts
from concourse.bass_utils import run_bass_kernel_spmd

N_CORES = 8
B, D_IN, D_OUT, N_CTRL = 8192, 512, 512, 8
B_SHARD = B // N_CORES          # 1024 rows per core
P = 128
N_TILES = B_SHARD // P          # 8 batch tiles per core
KS = D_IN // P                  # 4 K-chunks of 128
F32 = mybir.dt.float32
BF16 = mybir.dt.bfloat16

# Expert pacing groups: ordered so group g's experts are already loaded by
# the time its matmuls start (w_c lands ~2.9*(c+1) us on the gpsimd ring).
PACING = [[0], [1, 2], [3, 4], [5, 6, 7]]
N_WARM = 8


def _consts(nc: bass.Bass, const_pool):
    """One-time constants, embedded in the NEFF and DMA'd to SBUF: identity
    matrices for PE transposes.  identity_bf[:, c] doubles as the expert-
    selection vector: matmul(lhsT=identity_bf[:, c:c+1] broadcast to 128
    columns, rhs) replicates rhs row c onto all 128 output partitions."""
    import ml_dtypes

    identity_bf_d = nc.inline_tensor(
        np.eye(P, dtype=ml_dtypes.bfloat16), name="identity_bf_const"
    )
    identity_bf = const_pool.tile([P, P], BF16)
    nc.sync.dma_start(identity_bf, identity_bf_d.ap())

    identity_d = nc.inline_tensor(np.eye(P, dtype=np.float32), name="identity_const")
    identity = const_pool.tile([P, P], F32)
    nc.sync.dma_start(identity, identity_d.ap())
    return identity, identity_bf


def _body(nc: bass.Bass, tc: tile.TileContext, x_d, wt_d, w_d, b_d, o_d,
          identity, identity_bf, gate=None):
    with (
        tc.tile_pool(name="const", bufs=1) as const_pool,
        tc.tile_pool(name="xpool", bufs=4) as xpool,
        tc.tile_pool(name="xbfpool", bufs=3) as xbfpool,
        tc.tile_pool(name="xtpool", bufs=N_TILES) as xtpool,
        tc.tile_pool(name="xspool", bufs=2) as xspool,
        tc.tile_pool(name="opool", bufs=3) as opool,
        tc.tile_pool(name="ps", bufs=8, space="PSUM") as ps_pool,
    ):
        if gate is not None:
            # Serial-timing mode: the first DMA on each ring reads the
            # previous rep's final output tile (RAW dep) and writes back
            # values o_d already holds, serializing rep boundaries.
            nc.gpsimd.dma_start(o_d[0:1, 0:2], gate[0:1, 0:2])
            nc.scalar.dma_start(o_d[0:1, 2:4], gate[0:1, 2:4])
            nc.sync.dma_start(o_d[0:1, 4:6], gate[0:1, 4:6])

        # --- DMA issue order defines each ring's FIFO. ---
        # gpsimd SWDGE ring: the expert tensor only, cast f32->bf16 in
        # flight, one DMA per expert (each dma_start costs ~1us of SWDGE
        # descriptor generation, so fewer is better).
        # w_sb[p, c, k, o] = w[c, 128k+p, o].
        w_sb = const_pool.tile([P, N_CTRL, KS, D_OUT], BF16)
        for c in range(N_CTRL):
            nc.gpsimd.dma_start(
                w_sb[:, c], w_d[c].rearrange("(k p) o -> p k o", p=P)
            )

        # x tiles (f32; cast on ScalarE) in 3 batches — each dma_start holds
        # the issuing sequencer + HWDGE, so batch.  The first two batches go
        # on the sync ring (SP has nothing else to issue early, and the
        # scalar sequencer must stay free for the first casts); the tail
        # batch goes on the scalar ring.
        x_batches = []          # (first_t, ntiles, tile)
        for eng, t0b, nt in ((nc.scalar, 0, 2), (nc.scalar, 2, 2),
                             (nc.scalar, 4, 4)):
            x_f32 = xpool.tile([P, nt, D_IN], F32, tag=f"x_f32_{t0b}",
                               name=f"x_f32_{t0b}", bufs=2)
            eng.dma_start(
                x_f32,
                x_d[t0b * P:(t0b + nt) * P, :].rearrange(
                    "(t p) i -> p t i", p=P
                ),
            )
            x_batches.append((t0b, nt, x_f32))

        def x_f32_slice(t):
            for t0b, nt, x_f32 in x_batches:
                if t0b <= t < t0b + nt:
                    return x_f32[:, t - t0b, :]
            raise AssertionError

        # sync ring (behind the x batches): routing weights + bias (tiny).
        wt_nat = const_pool.tile([P, N_TILES, N_CTRL], F32)
        nc.sync.dma_start(wt_nat, wt_d.rearrange("(t p) c -> p t c", p=P))
        b_f32 = const_pool.tile([N_CTRL, D_OUT], F32)
        nc.sync.dma_start(b_f32, b_d)

        # --- ScalarE runs strictly in program order: the first two x casts
        # must lead so the first PE transposes aren't head-of-line blocked.
        x_bfs = {}
        for t in range(2):
            x_bf = xbfpool.tile([P, D_IN], BF16, tag="x_bf")
            nc.scalar.copy(x_bf, x_f32_slice(t))
            x_bfs[t] = x_bf

        # --- HAM warmup: N=512 dummy matmuls (broadcast rhs) keep PE busy
        # from ~0.4us so the clock-gate lifts early.  First allocation also
        # sizes the shared PSUM slots to a full bank.
        warm_rhs = identity_bf[:, None, :].to_broadcast([P, KS, P])
        for _ in range(N_WARM):
            warm_ps = ps_pool.tile([P, D_OUT], F32, tag="ps", name="warm_ps")
            nc.tensor.matmul(
                warm_ps, lhsT=identity_bf, rhs=warm_rhs,
                start=True, stop=True,
            )

        # --- X^T per tile: ScalarE cast to bf16, PE transpose (1 cyc/row),
        # ScalarE eviction.
        xts = []

        def transpose_tile(t):
            if t in x_bfs:
                x_bf = x_bfs[t]
            else:
                x_bf = xbfpool.tile([P, D_IN], BF16, tag="x_bf")
                nc.scalar.copy(x_bf, x_f32_slice(t))
            tr_ps = ps_pool.tile([P, KS, P], BF16, tag="ps")
            xt = xtpool.tile([P, KS, P], BF16)
            for k in range(KS):
                nc.tensor.transpose(tr_ps[:, k, :], x_bf[:, ts(k, P)],
                                    identity_bf)
            nc.scalar.copy(xt, tr_ps)
            xts.append(xt)

        transpose_tile(0)
        transpose_tile(1)

        # --- wt^T via PE transpose, zero-padded to 128 partitions (bf16);
        # wb[p, c, b] = weights[b, c] on every partition, via selection
        # matmuls: identity_bf[:, c] broadcast over the 128 lhsT columns
        # replicates wt_pad row c onto all output partitions.
        wt_pad = const_pool.tile([P, B_SHARD], BF16)
        nc.vector.memset(wt_pad, 0.0)
        wb = const_pool.tile([P, N_CTRL, B_SHARD], BF16)

        def wtt_half(h):
            for t in range(4 * h, 4 * h + 4):
                wtt_ps = ps_pool.tile([N_CTRL, P], F32, tag="ps")
                nc.tensor.transpose(wtt_ps, wt_nat[:, t, :], identity)
                nc.scalar.copy(wt_pad[0:N_CTRL, ts(t, P)], wtt_ps)

        def bc_expert(c, half=None):
            sel = identity_bf[:, c:c + 1].to_broadcast([P, P])
            for h in range(2) if half is None else (half,):
                bc_ps = ps_pool.tile([P, 512], F32, tag="ps")
                nc.tensor.matmul(
                    bc_ps, lhsT=sel, rhs=wt_pad[:, ts(h, 512)],
                    start=True, stop=True,
                )
                nc.scalar.copy(wb[:, c, ts(h, 512)], bc_ps)

        # First half of wb[c0] becomes available before the second half of
        # the wt transposes, so the first xs ops aren't gated on all of it.
        wtt_half(0)
        bc_expert(0, half=0)
        wtt_half(1)
        bc_expert(0, half=1)

        # Bias, zero-padded the same way.
        b_pad = const_pool.tile([P, D_OUT], BF16)
        nc.vector.memset(b_pad, 0.0)
        nc.vector.tensor_copy(b_pad[0:N_CTRL, :], b_f32)

        # --- Main accumulation: one PSUM bank per batch tile; bias first,
        # then all experts' K-chunks in pacing groups; single eviction.
        # Transposes of later x tiles and the remaining wb experts ride
        # inside the group-0 loop (PE would otherwise idle while group 0
        # is paced by the x stream).
        accs = [None] * N_TILES
        o_sb = None
        for gi, grp in enumerate(PACING):
            c0 = grp[0]
            cpg = len(grp)
            last_g = gi == len(PACING) - 1
            for t in range(N_TILES):
                if gi == 0 and t >= 2:
                    transpose_tile(t)
                if gi == 0 and 1 <= t < N_CTRL:
                    bc_expert(t)
                # xs[:, k, ci, :] = X^T * wb — layout [k, ci, b] keeps every
                # operand's innermost stride 1 so the DVE runs in 2x mode.
                xs = xspool.tile([P, KS, cpg, P], BF16, tag=f"xs{gi}", bufs=2)
                nc.vector.tensor_mul(
                    xs,
                    xts[t][:, :, None, :].to_broadcast([P, KS, cpg, P]),
                    wb[:, None, c0:c0 + cpg, ts(t, P)].to_broadcast(
                        [P, KS, cpg, P]
                    ),
                )
                if gi == 0:
                    accs[t] = ps_pool.tile([P, D_OUT], F32, tag="ps",
                                           name=f"acc{t}")
                    nc.tensor.matmul(
                        accs[t], lhsT=wt_pad[:, ts(t, P)], rhs=b_pad,
                        start=True, stop=False,
                    )
                for ci in range(cpg):
                    c = c0 + ci
                    for k in range(KS):
                        nc.tensor.matmul(
                            accs[t],
                            lhsT=xs[:, k, ci, :],
                            rhs=w_sb[:, c, k, :],
                            start=False,
                            stop=(last_g and ci == cpg - 1 and k == KS - 1),
                        )
                if last_g:
                    o_sb = opool.tile([P, D_OUT], F32, tag="o_sb")
                    if t == N_TILES - 1:
                        # Final tile: split eviction + store into halves so
                        # the out-DMA of the first half overlaps the second
                        # half's eviction (shrinks the kernel tail).
                        for h in range(2):
                            nc.scalar.copy(o_sb[:, ts(h, D_OUT // 2)],
                                           accs[t][:, ts(h, D_OUT // 2)])
                            nc.sync.dma_start(
                                o_d[ts(t, P), ts(h, D_OUT // 2)],
                                o_sb[:, ts(h, D_OUT // 2)],
                            )
                    else:
                        nc.scalar.copy(o_sb, accs[t])
                        nc.sync.dma_start(o_d[ts(t, P), :], o_sb)
        return o_sb


def _split_multi_waits(bir: dict) -> dict:
    """The walrus build in this container supports at most ONE sync-wait per
    instruction ("Too many sync wait commands" at codegen otherwise).  Tile's
    scheduler freely attaches several.  Split: keep the last wait on the
    instruction and hoist the others onto standalone same-engine
    EventSemaphore instructions inserted immediately before it — identical
    semantics (the engine blocks at the same program point)."""
    ctr = 0
    for func in bir["functions"]:
        for bb in func["blocks"]:
            new_insts = []
            for inst in bb["instructions"]:
                si = inst.get("sync_info")
                waits = si.get("on_wait") if si else None
                if waits and len(waits) > 1:
                    for w in waits[:-1]:
                        ctr += 1
                        new_insts.append(
                            {
                                "debug": inst.get("debug", 0),
                                "engine": inst["engine"],
                                "ins": [],
                                "outs": [],
                                "name": f"{inst['name']}-wsplit{ctr}",
                                "opcode": "EventSemaphore",
                                "sync_info": {"on_update": [], "on_wait": [w]},
                            }
                        )
                    si["on_wait"] = [waits[-1]]
                new_insts.append(inst)
            bb["instructions"] = new_insts
    return bir


_ORIG_TO_JSON_BYTES = bass.Bass.to_json_bytes


def _patched_to_json_bytes(self) -> bytes:
    bir = json.loads(_ORIG_TO_JSON_BYTES(self))
    _split_multi_waits(bir)
    return json.dumps(bir).encode()


_NC_CACHE = {}


def _build(reps: int = 1, serial: bool = False) -> bass.Bass:
    key = (reps, serial)
    if key in _NC_CACHE:
        return _NC_CACHE[key]
    nc = bass.Bass(
        "TRN2",
        target_bir_lowering=False,
        debug=False,
        enable_asserts=False,
        num_devices=N_CORES,
    )
    x_d = nc.dram_tensor("x_in", [B_SHARD, D_IN], F32, kind="ExternalInput").ap()
    wt_d = nc.dram_tensor("wt_in", [B_SHARD, N_CTRL], F32, kind="ExternalInput").ap()
    w_d = nc.dram_tensor("w_in", [N_CTRL, D_IN, D_OUT], F32, kind="ExternalInput").ap()
    b_d = nc.dram_tensor("b_in", [N_CTRL, D_OUT], F32, kind="ExternalInput").ap()
    o_d = nc.dram_tensor("out", [B_SHARD, D_OUT], F32, kind="ExternalOutput").ap()
    with tile.TileContext(nc) as tc:
        with tc.tile_pool(name="global_const", bufs=1) as gconst:
            identity, identity_bf = _consts(nc, gconst)
            gate = None
            for _ in range(reps):
                out_tile = _body(
                    nc, tc, x_d, wt_d, w_d, b_d, o_d, identity, identity_bf,
                    gate=gate,
                )
                if serial:
                    gate = out_tile
    nc.to_json_bytes = types.MethodType(_patched_to_json_bytes, nc)
    _NC_CACHE[key] = nc
    return nc


def kernel(inputs, weights, w, b, _trace=False, _reps=1, _serial=False):
    nc = _build(_reps, _serial)
    inputs = np.ascontiguousarray(inputs, dtype=np.float32)
    weights = np.ascontiguousarray(weights, dtype=np.float32)
    w = np.ascontiguousarray(w, dtype=np.float32)
    b = np.ascontiguousarray(b, dtype=np.float32)

    in_maps = []
    for i in range(N_CORES):
        sl = slice(i * B_SHARD, (i + 1) * B_SHARD)
        in_maps.append(
            {
                "x_in": inputs[sl],
                "wt_in": weights[sl],
                "w_in": w,
                "b_in": b,
            }
        )
    res = run_bass_kernel_spmd(
        nc, in_maps, core_ids=list(range(N_CORES)), trace=_trace
    )
    out = np.concatenate([r["out"] for r in res.results], axis=0)
    if _trace:
        return out, res
    return out


# revision 34
# speedup vs baseline: 1.6329x; 1.3260x over previous
"""Mode-adaptive linear (MoE soft routing) Trainium2 kernel.

out[b, o] = sum_c weights[b, c] * (inputs[b, :] @ w[c])[o] + (weights @ bias)[b, o]

Strategy: data-parallel shard of the batch across 8 NeuronCores (1024 rows
each); w/bias replicated.  Per core, routing weights are folded into the
transposed input tiles (xs = wb * X^T, bf16), and each 128-row batch tile
owns one PSUM bank that accumulates the bias matmul plus all 8 experts'
K-chunk matmuls (K = 8*512 + 8) before a single eviction.

Engine/ring budget per core:
  PE     : 264 N=512 bf16 matmuls (~31us) + 32 bf16 transposes + 16 wb
           broadcast matmuls + 8 wt transposes + HAM warmup
  DVE    : xs routing-weight scaling in 2x mode (~18us)
  Scalar : x f32->bf16 casts + PSUM evictions (~18us)
  gpsimd ring : the 8MB expert tensor, cast f32->bf16 in flight (w_c
           arrives ~2.9*(c+1) us -> expert pacing groups {1,2,2,3})
  scalar ring : x tile batches;  sync ring: identities/wt/b + output
"""

import json
import types

import numpy as np

import concourse.bass as bass
import concourse.mybir as mybir
import concourse.tile as tile
from concourse.bass import ts
from concourse.bass_utils import run_bass_kernel_spmd

N_CORES = 8
B, D_IN, D_OUT, N_CTRL = 8192, 512, 512, 8
B_SHARD = B // N_CORES          # 1024 rows per core
P = 128
N_TILES = B_SHARD // P          # 8 batch tiles per core
KS = D_IN // P                  # 4 K-chunks of 128
F32 = mybir.dt.float32
BF16 = mybir.dt.bfloat16

# Expert pacing groups: ordered so group g's experts are already loaded by
# the time its matmuls start (w_c lands ~2.9*(c+1) us on the gpsimd ring).
PACING = [[0], [1, 2], [3, 4], [5, 6, 7]]
N_WARM = 8


def _consts(nc: bass.Bass, const_pool):
    """One-time constants, embedded in the NEFF and DMA'd to SBUF: identity
    matrices for PE transposes.  identity_bf[:, c] doubles as the expert-
    selection vector: matmul(lhsT=identity_bf[:, c:c+1] broadcast to 128
    columns, rhs) replicates rhs row c onto all 128 output partitions."""
    import ml_dtypes

    identity_bf_d = nc.inline_tensor(
        np.eye(P, dtype=ml_dtypes.bfloat16), name="identity_bf_const"
    )
    identity_bf = const_pool.tile([P, P], BF16)
    nc.sync.dma_start(identity_bf, identity_bf_d.ap())

    identity_d = nc.inline_tensor(np.eye(P, dtype=np.float32), name="identity_const")
    identity = const_pool.tile([P, P], F32)
    nc.sync.dma_start(identity, identity_d.ap())
    return identity, identity_bf


def _body(nc: bass.Bass, tc: tile.TileContext, x_d, wt_d, w_d, b_d, o_d,
          identity, identity_bf, gate=None):
    with (
        tc.tile_pool(name="const", bufs=1) as const_pool,
        tc.tile_pool(name="xpool", bufs=4) as xpool,
        tc.tile_pool(name="xbfpool", bufs=3) as xbfpool,
        tc.tile_pool(name="xtpool", bufs=N_TILES) as xtpool,
        tc.tile_pool(name="xspool", bufs=2) as xspool,
        tc.tile_pool(name="opool", bufs=3) as opool,
        tc.tile_pool(name="ps", bufs=8, space="PSUM") as ps_pool,
    ):
        if gate is not None:
            # Serial-timing mode: the first DMA on each ring reads the
            # previous rep's final output tile (RAW dep) and writes back
            # values o_d already holds, serializing rep boundaries.
            nc.gpsimd.dma_start(o_d[0:1, 0:2], gate[0:1, 0:2])
            nc.scalar.dma_start(o_d[0:1, 2:4], gate[0:1, 2:4])
            nc.sync.dma_start(o_d[0:1, 4:6], gate[0:1, 4:6])

        # --- DMA issue order defines each ring's FIFO. ---
        # gpsimd SWDGE ring: the expert tensor only, cast f32->bf16 in
        # flight, one DMA per expert (each dma_start costs ~1us of SWDGE
        # descriptor generation, so fewer is better).
        # w_sb[p, c, k, o] = w[c, 128k+p, o].
        w_sb = const_pool.tile([P, N_CTRL, KS, D_OUT], BF16)
        for c in range(N_CTRL):
            nc.gpsimd.dma_start(
                w_sb[:, c], w_d[c].rearrange("(k p) o -> p k o", p=P)
            )

        # x tiles (f32; cast on ScalarE) in 3 batches — each dma_start holds
        # the issuing sequencer + HWDGE, so batch.  The first two batches go
        # on the sync ring (SP has nothing else to issue early, and the
        # scalar sequencer must stay free for the first casts); the tail
        # batch goes on the scalar ring.
        x_batches = []          # (first_t, ntiles, tile)
        for eng, t0b, nt in ((nc.scalar, 0, 2), (nc.scalar, 2, 2),
                             (nc.scalar, 4, 4)):
            x_f32 = xpool.tile([P, nt, D_IN], F32, tag=f"x_f32_{t0b}",
                               name=f"x_f32_{t0b}", bufs=2)
            eng.dma_start(
                x_f32,
                x_d[t0b * P:(t0b + nt) * P, :].rearrange(
                    "(t p) i -> p t i", p=P
                ),
            )
            x_batches.append((t0b, nt, x_f32))

        def x_f32_slice(t):
            for t0b, nt, x_f32 in x_batches:
                if t0b <= t < t0b + nt:
                    return x_f32[:, t - t0b, :]
            raise AssertionError

        # sync ring (behind the x batches): routing weights + bias (tiny).
        wt_nat = const_pool.tile([P, N_TILES, N_CTRL], F32)
        nc.sync.dma_start(wt_nat, wt_d.rearrange("(t p) c -> p t c", p=P))
        b_f32 = const_pool.tile([N_CTRL, D_OUT], F32)
        nc.sync.dma_start(b_f32, b_d)

        # --- ScalarE runs strictly in program order: the first two x casts
        # must lead so the first PE transposes aren't head-of-line blocked.
        x_bfs = {}
        for t in range(2):
            x_bf = xbfpool.tile([P, D_IN], BF16, tag="x_bf")
            nc.scalar.copy(x_bf, x_f32_slice(t))
            x_bfs[t] = x_bf

        # --- HAM warmup: N=512 dummy matmuls (broadcast rhs) keep PE busy
        # from ~0.4us so the clock-gate lifts early.  First allocation also
        # sizes the shared PSUM slots to a full bank.
        warm_rhs = identity_bf[:, None, :].to_broadcast([P, KS, P])
        for _ in range(N_WARM):
            warm_ps = ps_pool.tile([P, D_OUT], F32, tag="ps", name="warm_ps")
            nc.tensor.matmul(
                warm_ps, lhsT=identity_bf, rhs=warm_rhs,
                start=True, stop=True,
            )

        # --- X^T per tile: ScalarE cast to bf16, PE transpose (1 cyc/row),
        # ScalarE eviction.
        xts = []

        def transpose_tile(t):
            if t in x_bfs:
                x_bf = x_bfs[t]
            else:
                x_bf = xbfpool.tile([P, D_IN], BF16, tag="x_bf")
                nc.scalar.copy(x_bf, x_f32_slice(t))
            tr_ps = ps_pool.tile([P, KS, P], BF16, tag="ps")
            xt = xtpool.tile([P, KS, P], BF16)
            for k in range(KS):
                nc.tensor.transpose(tr_ps[:, k, :], x_bf[:, ts(k, P)],
                                    identity_bf)
            nc.scalar.copy(xt, tr_ps)
            xts.append(xt)

        transpose_tile(0)
        transpose_tile(1)

        # --- wt^T via PE transpose, zero-padded to 128 partitions (bf16);
        # wb[p, c, b] = weights[b, c] on every partition, via selection
        # matmuls: identity_bf[:, c] broadcast over the 128 lhsT columns
        # replicates wt_pad row c onto all output partitions.
        wt_pad = const_pool.tile([P, B_SHARD], BF16)
        nc.vector.memset(wt_pad, 0.0)
        wb = const_pool.tile([P, N_CTRL, B_SHARD], BF16)

        def wtt_half(h):
            for t in range(4 * h, 4 * h + 4):
                wtt_ps = ps_pool.tile([N_CTRL, P], F32, tag="ps")
                nc.tensor.transpose(wtt_ps, wt_nat[:, t, :], identity)
                nc.scalar.copy(wt_pad[0:N_CTRL, ts(t, P)], wtt_ps)

        def bc_expert(c, half=None):
            sel = identity_bf[:, c:c + 1].to_broadcast([P, P])
            for h in range(2) if half is None else (half,):
                bc_ps = ps_pool.tile([P, 512], F32, tag="ps")
                nc.tensor.matmul(
                    bc_ps, lhsT=sel, rhs=wt_pad[:, ts(h, 512)],
                    start=True, stop=True,
                )
                nc.scalar.copy(wb[:, c, ts(h, 512)], bc_ps)

        # First half of wb[c0] becomes available before the second half of
        # the wt transposes, so the first xs ops aren't gated on all of it.
        wtt_half(0)
        bc_expert(0, half=0)
        wtt_half(1)
        bc_expert(0, half=1)

        # Bias, zero-padded the same way.
        b_pad = const_pool.tile([P, D_OUT], BF16)
        nc.vector.memset(b_pad, 0.0)
        nc.vector.tensor_copy(b_pad[0:N_CTRL, :], b_f32)

        # --- Main accumulation: one PSUM bank per batch tile; bias first,
        # then all experts' K-chunks in pacing groups; single eviction.
        # Transposes of later x tiles and the remaining wb experts ride
        # inside the group-0 loop (PE would otherwise idle while group 0
        # is paced by the x stream).
        accs = [None] * N_TILES
        o_sb = None
        for gi, grp in enumerate(PACING):
            c0 = grp[0]
            cpg = len(grp)
            last_g = gi == len(PACING) - 1
            for t in range(N_TILES):
                if gi == 0 and t >= 2:
                    transpose_tile(t)
                if gi == 0 and 1 <= t < N_CTRL:
                    bc_expert(t)
                # xs[:, k, ci, :] = X^T * wb — layout [k, ci, b] keeps every
                # operand's innermost stride 1 so the DVE runs in 2x mode.
                xs = xspool.tile([P, KS, cpg, P], BF16, tag=f"xs{gi}", bufs=2)
                nc.vector.tensor_mul(
                    xs,
                    xts[t][:, :, None, :].to_broadcast([P, KS, cpg, P]),
                    wb[:, None, c0:c0 + cpg, ts(t, P)].to_broadcast(
                        [P, KS, cpg, P]
                    ),
                )
                if gi == 0:
                    accs[t] = ps_pool.tile([P, D_OUT], F32, tag="ps",
                                           name=f"acc{t}")
                    nc.tensor.matmul(
                        accs[t], lhsT=wt_pad[:, ts(t, P)], rhs=b_pad,
                        start=True, stop=False,
                    )
                for ci in range(cpg):
                    c = c0 + ci
                    for k in range(KS):
                        nc.tensor.matmul(
                            accs[t],
                            lhsT=xs[:, k, ci, :],
                            rhs=w_sb[:, c, k, :],
                            start=False,
                            stop=(last_g and ci == cpg - 1 and k == KS - 1),
                        )
                if last_g:
                    o_sb = opool.tile([P, D_OUT], F32, tag="o_sb")
                    if t == N_TILES - 1:
                        # Final tile: split eviction + store into halves so
                        # the out-DMA of the first half overlaps the second
                        # half's eviction (shrinks the kernel tail).
                        for h in range(2):
                            nc.scalar.copy(o_sb[:, ts(h, D_OUT // 2)],
                                           accs[t][:, ts(h, D_OUT // 2)])
                            nc.sync.dma_start(
                                o_d[ts(t, P), ts(h, D_OUT // 2)],
                                o_sb[:, ts(h, D_OUT // 2)],
                            )
                    else:
                        nc.scalar.copy(o_sb, accs[t])
                        nc.sync.dma_start(o_d[ts(t, P), :], o_sb)
        return o_sb


def _split_multi_waits(bir: dict) -> dict:
    """The walrus build in this container supports at most ONE sync-wait per
    instruction ("Too many sync wait commands" at codegen otherwise).  Tile's
    scheduler freely attaches several.  Split: keep the last wait on the
    instruction and hoist the others onto standalone same-engine
    EventSemaphore instructions inserted immediately before it — identical
    semantics (the engine blocks at the same program point)."""
    ctr = 0
    for func in bir["functions"]:
        for bb in func["blocks"]:
            new_insts = []
            for inst in bb["instructions"]:
                si = inst.get("sync_info")
                waits = si.get("on_wait") if si else None
                if waits and len(waits) > 1:
                    for w in waits[:-1]:
                        ctr += 1
                        new_insts.append(
                            {
                                "debug": inst.get("debug", 0),
                                "engine": inst["engine"],
                                "ins": [],
                                "outs": [],
                                "name": f"{inst['name']}-wsplit{ctr}",
                                "opcode": "EventSemaphore",
                                "sync_info": {"on_update": [], "on_wait": [w]},
                            }
                        )
                    si["on_wait"] = [waits[-1]]
                new_insts.append(inst)
            bb["instructions"] = new_insts
    return bir


_ORIG_TO_JSON_BYTES = bass.Bass.to_json_bytes


def _patched_to_json_bytes(self) -> bytes:
    bir = json.loads(_ORIG_TO_JSON_BYTES(self))
    _split_multi_waits(bir)
    return json.dumps(bir).encode()


_NC_CACHE = {}


def _build(reps: int = 1, serial: bool = False) -> bass.Bass:
    key = (reps, serial)
    if key in _NC_CACHE:
        return _NC_CACHE[key]
    nc = bass.Bass(
        "TRN2",
        target_bir_lowering=False,
        debug=False,
        enable_asserts=False,
        num_devices=N_CORES,
    )
    x_d = nc.dram_tensor("x_in", [B_SHARD, D_IN], F32, kind="ExternalInput").ap()
    wt_d = nc.dram_tensor("wt_in", [B_SHARD, N_CTRL], F32, kind="ExternalInput").ap()
    w_d = nc.dram_tensor("w_in", [N_CTRL, D_IN, D_OUT], F32, kind="ExternalInput").ap()
    b_d = nc.dram_tensor("b_in", [N_CTRL, D_OUT], F32, kind="ExternalInput").ap()
    o_d = nc.dram_tensor("out", [B_SHARD, D_OUT], F32, kind="ExternalOutput").ap()
    with tile.TileContext(nc) as tc:
        with tc.tile_pool(name="global_const", bufs=1) as gconst:
            identity, identity_bf = _consts(nc, gconst)
            gate = None
            for _ in range(reps):
                out_tile = _body(
                    nc, tc, x_d, wt_d, w_d, b_d, o_d, identity, identity_bf,
                    gate=gate,
                )
                if serial:
                    gate = out_tile
    nc.to_json_bytes = types.MethodType(_patched_to_json_bytes, nc)
    _NC_CACHE[key] = nc
    return nc


def kernel(inputs, weights, w, b, _trace=False, _reps=1, _serial=False):
    nc = _build(_reps, _serial)
    inputs = np.ascontiguousarray(inputs, dtype=np.float32)
    weights = np.ascontiguousarray(weights, dtype=np.float32)
    w = np.ascontiguousarray(w, dtype=np.float32)
    b = np.ascontiguousarray(b, dtype=np.float32)

    in_maps = []
    for i in range(N_CORES):
        sl = slice(i * B_SHARD, (i + 1) * B_SHARD)
        in_maps.append(
            {
                "x_in": inputs[sl],
                "wt_in": weights[sl],
                "w_in": w,
                "b_in": b,
            }
        )
    res = run_bass_kernel_spmd(
        nc, in_maps, core_ids=list(range(N_CORES)), trace=_trace
    )
    out = np.concatenate([r["out"] for r in res.results], axis=0)
    if _trace:
        return out, res
    return out


# revision 41
# speedup vs baseline: 1.9733x; 1.2085x over previous
"""Mode-adaptive linear (MoE soft routing) Trainium2 kernel.

out[b, o] = sum_c weights[b, c] * (inputs[b, :] @ w[c])[o] + (weights @ bias)[b, o]

Strategy: data-parallel shard of the batch across 8 NeuronCores (1024 rows
each); w/bias replicated.  Per core, routing weights are folded into the
transposed input tiles (xs = wb * X^T, bf16), and each 128-row batch tile
owns one PSUM bank that accumulates the bias matmul plus all 8 experts'
K-chunk matmuls (K = 8*512 + 8) before a single eviction.

Engine/ring budget per core:
  PE     : 264 N=512 bf16 matmuls (~31us) + 32 bf16 transposes + 16 wb
           broadcast matmuls + 8 wt transposes + HAM warmup
  DVE    : xs routing-weight scaling in 2x mode (~18us)
  Scalar : x f32->bf16 casts + PSUM evictions (~18us)
  gpsimd ring : the 8MB expert tensor, cast f32->bf16 in flight (w_c
           arrives ~2.9*(c+1) us -> expert pacing groups {1,2,2,3})
  scalar ring : x tile batches;  sync ring: identities/wt/b + output
"""

import json
import types

import numpy as np

import concourse.bass as bass
import concourse.mybir as mybir
import concourse.tile as tile
from concourse.bass import ts
from concourse.bass_utils import run_bass_kernel_spmd

N_CORES = 8
B, D_IN, D_OUT, N_CTRL = 8192, 512, 512, 8
B_SHARD = B // N_CORES          # 1024 rows per core
P = 128
N_TILES = B_SHARD // P          # 8 batch tiles per core
KS = D_IN // P                  # 4 K-chunks of 128
F32 = mybir.dt.float32
BF16 = mybir.dt.bfloat16

# Expert pacing groups: ordered so group g's experts are already loaded by
# the time its matmuls start (w_c lands ~2.9*(c+1) us on the gpsimd ring).
PACING = [[0], [1, 2], [3, 4], [5, 6, 7]]
N_WARM = 8


def _consts(nc: bass.Bass, const_pool):
    """One-time constants, embedded in the NEFF and DMA'd to SBUF: identity
    matrices for PE transposes.  identity_bf[:, c] doubles as the expert-
    selection vector: matmul(lhsT=identity_bf[:, c:c+1] broadcast to 128
    columns, rhs) replicates rhs row c onto all 128 output partitions."""
    import ml_dtypes

    identity_bf_d = nc.inline_tensor(
        np.eye(P, dtype=ml_dtypes.bfloat16), name="identity_bf_const"
    )
    identity_bf = const_pool.tile([P, P], BF16)
    nc.sync.dma_start(identity_bf, identity_bf_d.ap())

    identity_d = nc.inline_tensor(np.eye(P, dtype=np.float32), name="identity_const")
    identity = const_pool.tile([P, P], F32)
    nc.sync.dma_start(identity, identity_d.ap())
    return identity, identity_bf


def _body(nc: bass.Bass, tc: tile.TileContext, x_d, wt_d, w_d, b_d, o_d,
          identity, identity_bf, gate=None):
    with (
        tc.tile_pool(name="const", bufs=1) as const_pool,
        tc.tile_pool(name="xpool", bufs=4) as xpool,
        tc.tile_pool(name="xbfpool", bufs=3) as xbfpool,
        tc.tile_pool(name="xtpool", bufs=N_TILES) as xtpool,
        tc.tile_pool(name="xspool", bufs=2) as xspool,
        tc.tile_pool(name="opool", bufs=3) as opool,
        tc.tile_pool(name="ps", bufs=8, space="PSUM") as ps_pool,
    ):
        if gate is not None:
            # Serial-timing mode: the first DMA on each ring reads the
            # previous rep's final output tile (RAW dep) and writes back
            # values o_d already holds, serializing rep boundaries.
            nc.gpsimd.dma_start(o_d[0:1, 0:2], gate[0:1, 0:2])
            nc.scalar.dma_start(o_d[0:1, 2:4], gate[0:1, 2:4])
            nc.sync.dma_start(o_d[0:1, 4:6], gate[0:1, 4:6])

        # --- DMA issue order defines each ring's FIFO. ---
        # gpsimd SWDGE ring: the expert tensor only, cast f32->bf16 in
        # flight, one DMA per expert (each dma_start costs ~1us of SWDGE
        # descriptor generation, so fewer is better).
        # w_sb[p, c, k, o] = w[c, 128k+p, o].
        w_sb = const_pool.tile([P, N_CTRL, KS, D_OUT], BF16)
        # expert 0 in two halves so its first K-chunks land ~1.5us earlier
        # (the very first expert matmuls are gated on them)
        for h in range(2):
            nc.gpsimd.dma_start(
                w_sb[:, 0, ts(h, KS // 2)],
                w_d[0, ts(h, D_IN // 2)].rearrange("(k p) o -> p k o", p=P),
            )
        for c in range(1, N_CTRL):
            nc.gpsimd.dma_start(
                w_sb[:, c], w_d[c].rearrange("(k p) o -> p k o", p=P)
            )

        # x tiles (f32; cast on ScalarE) in 3 batches — each dma_start holds
        # the issuing sequencer + HWDGE, so batch.  The first two batches go
        # on the sync ring (SP has nothing else to issue early, and the
        # scalar sequencer must stay free for the first casts); the tail
        # batch goes on the scalar ring.
        x_batches = []          # (first_t, ntiles, tile)
        for eng, t0b, nt in ((nc.scalar, 0, 1), (nc.scalar, 1, 3),
                             (nc.scalar, 4, 4)):
            x_f32 = xpool.tile([P, nt, D_IN], F32, tag=f"x_f32_{t0b}",
                               name=f"x_f32_{t0b}", bufs=2)
            eng.dma_start(
                x_f32,
                x_d[t0b * P:(t0b + nt) * P, :].rearrange(
                    "(t p) i -> p t i", p=P
                ),
            )
            x_batches.append((t0b, nt, x_f32))

        def x_f32_slice(t):
            for t0b, nt, x_f32 in x_batches:
                if t0b <= t < t0b + nt:
                    return x_f32[:, t - t0b, :]
            raise AssertionError

        # sync ring (behind the x batches): routing weights + bias (tiny).
        wt_nat = const_pool.tile([P, N_TILES, N_CTRL], F32)
        nc.sync.dma_start(wt_nat, wt_d.rearrange("(t p) c -> p t c", p=P))
        b_f32 = const_pool.tile([N_CTRL, D_OUT], F32)
        nc.sync.dma_start(b_f32, b_d)

        # --- ScalarE runs strictly in program order: the first two x casts
        # must lead so the first PE transposes aren't head-of-line blocked.
        x_bfs = {}
        for t in range(2):
            x_bf = xbfpool.tile([P, D_IN], BF16, tag="x_bf")
            nc.scalar.copy(x_bf, x_f32_slice(t))
            x_bfs[t] = x_bf

        # --- HAM warmup: N=512 dummy matmuls (broadcast rhs) keep PE busy
        # from ~0.4us so the clock-gate lifts early.  First allocation also
        # sizes the shared PSUM slots to a full bank.
        warm_rhs = identity_bf[:, None, :].to_broadcast([P, KS, P])
        for _ in range(N_WARM):
            warm_ps = ps_pool.tile([P, D_OUT], F32, tag="ps", name="warm_ps")
            nc.tensor.matmul(
                warm_ps, lhsT=identity_bf, rhs=warm_rhs,
                start=True, stop=True,
            )

        # --- X^T per tile: ScalarE cast to bf16, PE transpose (1 cyc/row),
        # ScalarE eviction.
        xts = []

        def transpose_tile(t):
            if t in x_bfs:
                x_bf = x_bfs[t]
            else:
                x_bf = xbfpool.tile([P, D_IN], BF16, tag="x_bf")
                nc.scalar.copy(x_bf, x_f32_slice(t))
            tr_ps = ps_pool.tile([P, KS, P], BF16, tag="ps")
            xt = xtpool.tile([P, KS, P], BF16)
            for k in range(KS):
                nc.tensor.transpose(tr_ps[:, k, :], x_bf[:, ts(k, P)],
                                    identity_bf)
            nc.scalar.copy(xt, tr_ps)
            xts.append(xt)

        transpose_tile(0)
        transpose_tile(1)

        # --- wt^T via PE transpose, zero-padded to 128 partitions (bf16);
        # wb[p, c, b] = weights[b, c] on every partition, via selection
        # matmuls: identity_bf[:, c] broadcast over the 128 lhsT columns
        # replicates wt_pad row c onto all output partitions.
        wt_pad = const_pool.tile([P, B_SHARD], BF16)
        nc.vector.memset(wt_pad, 0.0)
        wb = const_pool.tile([P, N_CTRL, B_SHARD], BF16)

        def wtt_half(h):
            for t in range(4 * h, 4 * h + 4):
                wtt_ps = ps_pool.tile([N_CTRL, P], F32, tag="ps")
                nc.tensor.transpose(wtt_ps, wt_nat[:, t, :], identity)
                nc.scalar.copy(wt_pad[0:N_CTRL, ts(t, P)], wtt_ps)

        def bc_expert(c, half=None):
            sel = identity_bf[:, c:c + 1].to_broadcast([P, P])
            for h in range(2) if half is None else (half,):
                bc_ps = ps_pool.tile([P, 512], F32, tag="ps")
                nc.tensor.matmul(
                    bc_ps, lhsT=sel, rhs=wt_pad[:, ts(h, 512)],
                    start=True, stop=True,
                )
                nc.scalar.copy(wb[:, c, ts(h, 512)], bc_ps)

        # First half of wb[c0] becomes available before the second half of
        # the wt transposes, so the first xs ops aren't gated on all of it.
        wtt_half(0)
        bc_expert(0, half=0)
        wtt_half(1)
        bc_expert(0, half=1)

        # Bias, zero-padded the same way.
        b_pad = const_pool.tile([P, D_OUT], BF16)
        nc.vector.memset(b_pad, 0.0)
        nc.vector.tensor_copy(b_pad[0:N_CTRL, :], b_f32)

        # --- Main accumulation: one PSUM bank per batch tile; bias first,
        # then all experts' K-chunks in pacing groups; single eviction.
        # Transposes of later x tiles and the remaining wb experts ride
        # inside the group-0 loop (PE would otherwise idle while group 0
        # is paced by the x stream).
        accs = [None] * N_TILES
        o_sb = None
        for gi, grp in enumerate(PACING):
            c0 = grp[0]
            cpg = len(grp)
            last_g = gi == len(PACING) - 1
            for t in range(N_TILES):
                if gi == 0 and t >= 2:
                    transpose_tile(t)
                if gi == 0 and 1 <= t < N_CTRL:
                    bc_expert(t)
                # xs[:, k, ci, :] = X^T * wb — layout [k, ci, b] keeps every
                # operand's innermost stride 1 so the DVE runs in 2x mode.
                xs = xspool.tile([P, KS, cpg, P], BF16, tag=f"xs{gi}", bufs=2)
                nc.vector.tensor_mul(
                    xs,
                    xts[t][:, :, None, :].to_broadcast([P, KS, cpg, P]),
                    wb[:, None, c0:c0 + cpg, ts(t, P)].to_broadcast(
                        [P, KS, cpg, P]
                    ),
                )
                if gi == 0:
                    accs[t] = ps_pool.tile([P, D_OUT], F32, tag="ps",
                                           name=f"acc{t}")
                    nc.tensor.matmul(
                        accs[t], lhsT=wt_pad[:, ts(t, P)], rhs=b_pad,
                        start=True, stop=False,
                    )
                for ci in range(cpg):
                    c = c0 + ci
                    for k in range(KS):
                        nc.tensor.matmul(
                            accs[t],
                            lhsT=xs[:, k, ci, :],
                            rhs=w_sb[:, c, k, :],
                            start=False,
                            stop=(last_g and ci == cpg - 1 and k == KS - 1),
                        )
                if last_g:
                    o_sb = opool.tile([P, D_OUT], F32, tag="o_sb")
                    if t == N_TILES - 1:
                        # Final tile: split eviction + store into halves so
                        # the out-DMA of the first half overlaps the second
                        # half's eviction (shrinks the kernel tail).
                        for h in range(2):
                            nc.scalar.copy(o_sb[:, ts(h, D_OUT // 2)],
                                           accs[t][:, ts(h, D_OUT // 2)])
                            nc.sync.dma_start(
                                o_d[ts(t, P), ts(h, D_OUT // 2)],
                                o_sb[:, ts(h, D_OUT // 2)],
                            )
                    else:
                        nc.scalar.copy(o_sb, accs[t])
                        nc.sync.dma_start(o_d[ts(t, P), :], o_sb)
        return o_sb


def _split_multi_waits(bir: dict) -> dict:
    """The walrus build in this container supports at most ONE sync-wait per
    instruction ("Too many sync wait commands" at codegen otherwise).  Tile's
    scheduler freely attaches several.  Split: keep the last wait on the
    instruction and hoist the others onto standalone same-engine
    EventSemaphore instructions inserted immediately before it — identical
    semantics (the engine blocks at the same program point)."""
    ctr = 0
    for func in bir["functions"]:
        for bb in func["blocks"]:
            new_insts = []
            for inst in bb["instructions"]:
                si = inst.get("sync_info")
                waits = si.get("on_wait") if si else None
                if waits and len(waits) > 1:
                    for w in waits[:-1]:
                        ctr += 1
                        new_insts.append(
                            {
                                "debug": inst.get("debug", 0),
                                "engine": inst["engine"],
                                "ins": [],
                                "outs": [],
                                "name": f"{inst['name']}-wsplit{ctr}",
                                "opcode": "EventSemaphore",
                                "sync_info": {"on_update": [], "on_wait": [w]},
                            }
                        )
                    si["on_wait"] = [waits[-1]]
                new_insts.append(inst)
            bb["instructions"] = new_insts
    return bir


_ENGINE_SEM_PREFIXES = ("PE_", "DVE_", "Activation_", "SP_", "Pool_")


def _strip_redundant_updates(bir: dict) -> dict:
    """Tile attaches a sem update to EVERY engine instruction, but sem-inc
    register writes serialize (~26 ns each) — ~8.5 us of pure PE tail here.
    Engine instructions complete strictly in program order, so an update is
    only needed where some wait actually tests that value.  Keep exactly the
    updates whose cumulative count is waited on (plus the final one per sem,
    for the end-of-program barrier) and renumber all wait thresholds.
    DMA sems (DMAHW*/DMASW*) complete out of program order — untouched."""
    for func in bir["functions"]:
        insts = [i for bb in func["blocks"] for i in bb["instructions"]]
        updates = {}   # sem id -> list of update dicts in program order
        waited = {}    # sem id -> set of waited values
        names = {}     # sem id -> ant_name
        odd_modes = set()   # sems with non-standard update/wait modes
        for inst in insts:
            si = inst.get("sync_info")
            if not si:
                continue
            for u in si.get("on_update") or []:
                names[u["id"]] = u.get("ant_name", "")
                updates.setdefault(u["id"], []).append(u)
                if u.get("update_mode") != "sem-inc":
                    odd_modes.add(u["id"])
            for w in si.get("on_wait") or []:
                names[w["id"]] = w.get("ant_name", "")
                waited.setdefault(w["id"], set()).add(w["wait_value"])
                if w.get("wait_mode") != "sem-ge-imm":
                    odd_modes.add(w["id"])
        remap = {}     # sem id -> {old value -> new value}
        drop = set()   # ids of update dicts to remove
        for sem, ups in updates.items():
            name = names.get(sem, "")
            if not name.startswith(_ENGINE_SEM_PREFIXES):
                continue
            if sem in odd_modes:
                continue
            if any(u.get("update_value") not in (None, 1) for u in ups):
                continue
            if any(not (1 <= v <= len(ups)) for v in waited.get(sem, ())):
                continue
            keep_idx = {len(ups) - 1}
            for v in waited.get(sem, ()):  # v-th update makes count reach v
                if 1 <= v <= len(ups):
                    keep_idx.add(v - 1)
            vmap = {}
            new_count = 0
            for i, u in enumerate(ups):
                if i in keep_idx:
                    new_count += 1
                else:
                    drop.add(id(u))
                vmap[i + 1] = new_count
            remap[sem] = vmap
        for inst in insts:
            si = inst.get("sync_info")
            if not si:
                continue
            ups = si.get("on_update")
            if ups:
                si["on_update"] = [u for u in ups if id(u) not in drop]
            for w in si.get("on_wait") or []:
                vmap = remap.get(w["id"])
                if vmap and w["wait_value"] in vmap:
                    w["wait_value"] = vmap[w["wait_value"]]
    return bir


_ORIG_TO_JSON_BYTES = bass.Bass.to_json_bytes


def _patched_to_json_bytes(self) -> bytes:
    bir = json.loads(_ORIG_TO_JSON_BYTES(self))
    _strip_redundant_updates(bir)
    _split_multi_waits(bir)
    return json.dumps(bir).encode()


_NC_CACHE = {}


def _build(reps: int = 1, serial: bool = False) -> bass.Bass:
    key = (reps, serial)
    if key in _NC_CACHE:
        return _NC_CACHE[key]
    nc = bass.Bass(
        "TRN2",
        target_bir_lowering=False,
        debug=False,
        enable_asserts=False,
        num_devices=N_CORES,
    )
    x_d = nc.dram_tensor("x_in", [B_SHARD, D_IN], F32, kind="ExternalInput").ap()
    wt_d = nc.dram_tensor("wt_in", [B_SHARD, N_CTRL], F32, kind="ExternalInput").ap()
    w_d = nc.dram_tensor("w_in", [N_CTRL, D_IN, D_OUT], F32, kind="ExternalInput").ap()
    b_d = nc.dram_tensor("b_in", [N_CTRL, D_OUT], F32, kind="ExternalInput").ap()
    o_d = nc.dram_tensor("out", [B_SHARD, D_OUT], F32, kind="ExternalOutput").ap()
    with tile.TileContext(nc) as tc:
        with tc.tile_pool(name="global_const", bufs=1) as gconst:
            identity, identity_bf = _consts(nc, gconst)
            gate = None
            for _ in range(reps):
                out_tile = _body(
                    nc, tc, x_d, wt_d, w_d, b_d, o_d, identity, identity_bf,
                    gate=gate,
                )
                if serial:
                    gate = out_tile
    nc.to_json_bytes = types.MethodType(_patched_to_json_bytes, nc)
    _NC_CACHE[key] = nc
    return nc


def kernel(inputs, weights, w, b, _trace=False, _reps=1, _serial=False):
    nc = _build(_reps, _serial)
    inputs = np.ascontiguousarray(inputs, dtype=np.float32)
    weights = np.ascontiguousarray(weights, dtype=np.float32)
    w = np.ascontiguousarray(w, dtype=np.float32)
    b = np.ascontiguousarray(b, dtype=np.float32)

    in_maps = []
    for i in range(N_CORES):
        sl = slice(i * B_SHARD, (i + 1) * B_SHARD)
        in_maps.append(
            {
                "x_in": inputs[sl],
                "wt_in": weights[sl],
                "w_in": w,
                "b_in": b,
            }
        )
    res = run_bass_kernel_spmd(
        nc, in_maps, core_ids=list(range(N_CORES)), trace=_trace
    )
    out = np.concatenate([r["out"] for r in res.results], axis=0)
    if _trace:
        return out, res
    return out


# revision 51
# speedup vs baseline: 2.0798x; 1.0539x over previous
"""Mode-adaptive linear (MoE soft routing) Trainium2 kernel.

out[b, o] = sum_c weights[b, c] * (inputs[b, :] @ w[c])[o] + (weights @ bias)[b, o]

Strategy: data-parallel shard of the batch across 8 NeuronCores (1024 rows
each); w/bias replicated.  Per core, routing weights are folded into the
transposed input tiles (xs = wb * X^T, bf16), and each 128-row batch tile
owns one PSUM bank that accumulates the bias matmul plus all 8 experts'
K-chunk matmuls (K = 8*512 + 8) before a single eviction.

Engine/ring budget per core:
  PE     : 264 N=512 bf16 matmuls (~31us) + 32 bf16 transposes + 16 wb
           broadcast matmuls + 8 wt transposes + HAM warmup
  DVE    : xs routing-weight scaling in 2x mode (~18us)
  Scalar : x f32->bf16 casts + PSUM evictions (~18us)
  gpsimd ring : the 8MB expert tensor, cast f32->bf16 in flight (w_c
           arrives ~2.9*(c+1) us -> expert pacing groups {1,2,2,3})
  scalar ring : x tile batches;  sync ring: identities/wt/b + output
"""

import json
import types

import numpy as np

import concourse.bass as bass
import concourse.mybir as mybir
import concourse.tile as tile
from concourse.bass import ts
from concourse.bass_utils import run_bass_kernel_spmd

N_CORES = 8
B, D_IN, D_OUT, N_CTRL = 8192, 512, 512, 8
B_SHARD = B // N_CORES          # 1024 rows per core
P = 128
N_TILES = B_SHARD // P          # 8 batch tiles per core
KS = D_IN // P                  # 4 K-chunks of 128
F32 = mybir.dt.float32
BF16 = mybir.dt.bfloat16

# Expert pacing groups: ordered so group g's experts are already loaded by
# the time its matmuls start (w_c lands ~2.9*(c+1) us on the gpsimd ring).
PACING = [[0], [1, 2], [3, 4], [5, 6, 7]]
N_WARM = 8


def _consts(nc: bass.Bass, const_pool):
    """One-time constants, embedded in the NEFF and DMA'd to SBUF: identity
    matrices for PE transposes.  identity_bf[:, c] doubles as the expert-
    selection vector: matmul(lhsT=identity_bf[:, c:c+1] broadcast to 128
    columns, rhs) replicates rhs row c onto all 128 output partitions."""
    import ml_dtypes

    identity_bf_d = nc.inline_tensor(
        np.eye(P, dtype=ml_dtypes.bfloat16), name="identity_bf_const"
    )
    identity_bf = const_pool.tile([P, P], BF16)
    nc.sync.dma_start(identity_bf, identity_bf_d.ap())
    return identity_bf


def _body(nc: bass.Bass, tc: tile.TileContext, x_d, wt_d, w_d, b_d, o_d,
          identity_bf, gate=None):
    with (
        tc.tile_pool(name="const", bufs=1) as const_pool,
        tc.tile_pool(name="xpool", bufs=4) as xpool,
        tc.tile_pool(name="xbfpool", bufs=3) as xbfpool,
        tc.tile_pool(name="xtpool", bufs=N_TILES) as xtpool,
        tc.tile_pool(name="xspool", bufs=2) as xspool,
        tc.tile_pool(name="opool", bufs=3) as opool,
        tc.tile_pool(name="ps", bufs=8, space="PSUM") as ps_pool,
    ):
        if gate is not None:
            # Serial-timing mode: the first DMA on each ring reads the
            # previous rep's final output tile (RAW dep) and writes back
            # values o_d already holds, serializing rep boundaries.
            nc.gpsimd.dma_start(o_d[0:1, 0:2], gate[0:1, 0:2])
            nc.scalar.dma_start(o_d[0:1, 2:4], gate[0:1, 2:4])
            nc.sync.dma_start(o_d[0:1, 4:6], gate[0:1, 4:6])

        # --- DMA issue order defines each ring's FIFO. ---
        # gpsimd SWDGE ring: the expert tensor only, cast f32->bf16 in
        # flight, one DMA per expert (each dma_start costs ~1us of SWDGE
        # descriptor generation, so fewer is better).
        # w_sb[p, c, k, o] = w[c, 128k+p, o].
        w_sb = const_pool.tile([P, N_CTRL, KS, D_OUT], BF16)
        # expert 0 in two halves so its first K-chunks land ~1.5us earlier
        # (the very first expert matmuls are gated on them)
        for h in range(2):
            nc.gpsimd.dma_start(
                w_sb[:, 0, ts(h, KS // 2)],
                w_d[0, ts(h, D_IN // 2)].rearrange("(k p) o -> p k o", p=P),
            )
        for c in range(1, N_CTRL):
            nc.gpsimd.dma_start(
                w_sb[:, c], w_d[c].rearrange("(k p) o -> p k o", p=P)
            )

        # x tiles (f32; cast on ScalarE) in 3 batches — each dma_start holds
        # the issuing sequencer + HWDGE, so batch.  The first two batches go
        # on the sync ring (SP has nothing else to issue early, and the
        # scalar sequencer must stay free for the first casts); the tail
        # batch goes on the scalar ring.
        x_batches = []          # (first_t, ntiles, tile)
        for eng, t0b, nt in ((nc.scalar, 0, 1), (nc.scalar, 1, 3),
                             (nc.scalar, 4, 4)):
            x_f32 = xpool.tile([P, nt, D_IN], F32, tag=f"x_f32_{t0b}",
                               name=f"x_f32_{t0b}", bufs=2)
            eng.dma_start(
                x_f32,
                x_d[t0b * P:(t0b + nt) * P, :].rearrange(
                    "(t p) i -> p t i", p=P
                ),
            )
            x_batches.append((t0b, nt, x_f32))

        def x_f32_slice(t):
            for t0b, nt, x_f32 in x_batches:
                if t0b <= t < t0b + nt:
                    return x_f32[:, t - t0b, :]
            raise AssertionError

        # sync ring (behind the x batches): routing weights + bias (tiny).
        wt_nat = const_pool.tile([P, N_TILES, N_CTRL], F32)
        nc.sync.dma_start(wt_nat, wt_d.rearrange("(t p) c -> p t c", p=P))
        b_f32 = const_pool.tile([N_CTRL, D_OUT], F32)
        nc.sync.dma_start(b_f32, b_d)

        # --- ScalarE runs strictly in program order: the first two x casts
        # must lead so the first PE transposes aren't head-of-line blocked.
        x_bfs = {}
        for t in range(2):
            x_bf = xbfpool.tile([P, D_IN], BF16, tag="x_bf")
            nc.scalar.copy(x_bf, x_f32_slice(t))
            x_bfs[t] = x_bf

        # --- HAM warmup: N=512 dummy matmuls (broadcast rhs) keep PE busy
        # from ~0.4us so the clock-gate lifts early.  First allocation also
        # sizes the shared PSUM slots to a full bank.
        warm_rhs = identity_bf[:, None, :].to_broadcast([P, KS, P])
        for _ in range(N_WARM):
            warm_ps = ps_pool.tile([P, D_OUT], F32, tag="ps", name="warm_ps")
            nc.tensor.matmul(
                warm_ps, lhsT=identity_bf, rhs=warm_rhs,
                start=True, stop=True,
            )

        # --- X^T per tile: ScalarE cast to bf16, PE transpose (1 cyc/row),
        # ScalarE eviction.
        xts = []

        def transpose_tile(t):
            if t in x_bfs:
                x_bf = x_bfs[t]
            else:
                x_bf = xbfpool.tile([P, D_IN], BF16, tag="x_bf")
                nc.scalar.copy(x_bf, x_f32_slice(t))
            tr_ps = ps_pool.tile([P, KS, P], BF16, tag="ps")
            xt = xtpool.tile([P, KS, P], BF16)
            for k in range(KS):
                nc.tensor.transpose(tr_ps[:, k, :], x_bf[:, ts(k, P)],
                                    identity_bf)
            nc.scalar.copy(xt, tr_ps)
            xts.append(xt)

        transpose_tile(0)
        transpose_tile(1)

        # --- wt^T via PE transpose, zero-padded to 128 partitions (bf16);
        # wb[p, c, b] = weights[b, c] on every partition, via selection
        # matmuls: identity_bf[:, c] broadcast over the 128 lhsT columns
        # replicates wt_pad row c onto all output partitions.
        wt_pad = const_pool.tile([P, B_SHARD], BF16)
        nc.vector.memset(wt_pad, 0.0)
        wb = const_pool.tile([P, N_CTRL, B_SHARD], BF16)

        # wt in bf16 first (numerically identical — the eviction cast to
        # bf16 happens either way), so the transposes run at 1 cyc/row.
        wt_bf = const_pool.tile([P, N_TILES, N_CTRL], BF16)
        nc.vector.tensor_copy(wt_bf, wt_nat)

        def wtt_half(h):
            for t in range(4 * h, 4 * h + 4):
                wtt_ps = ps_pool.tile([N_CTRL, P], BF16, tag="ps")
                nc.tensor.transpose(wtt_ps, wt_bf[:, t, :], identity_bf)
                nc.scalar.copy(wt_pad[0:N_CTRL, ts(t, P)], wtt_ps)

        def bc_expert(c, half=None):
            sel = identity_bf[:, c:c + 1].to_broadcast([P, P])
            for h in range(2) if half is None else (half,):
                bc_ps = ps_pool.tile([P, 512], F32, tag="ps")
                nc.tensor.matmul(
                    bc_ps, lhsT=sel, rhs=wt_pad[:, ts(h, 512)],
                    start=True, stop=True,
                )
                nc.scalar.copy(wb[:, c, ts(h, 512)], bc_ps)

        # First half of wb[c0] becomes available before the second half of
        # the wt transposes, so the first xs ops aren't gated on all of it.
        wtt_half(0)
        bc_expert(0, half=0)
        wtt_half(1)
        bc_expert(0, half=1)

        # Bias, zero-padded the same way.
        b_pad = const_pool.tile([P, D_OUT], BF16)
        nc.vector.memset(b_pad, 0.0)
        nc.vector.tensor_copy(b_pad[0:N_CTRL, :], b_f32)

        # --- Main accumulation: one PSUM bank per batch tile; bias first,
        # then all experts' K-chunks in pacing groups; single eviction.
        # Transposes of later x tiles and the remaining wb experts ride
        # inside the group-0 loop (PE would otherwise idle while group 0
        # is paced by the x stream).
        accs = [None] * N_TILES
        o_sb = None
        for gi, grp in enumerate(PACING):
            c0 = grp[0]
            cpg = len(grp)
            last_g = gi == len(PACING) - 1
            for t in range(N_TILES):
                if gi == 0 and t >= 2:
                    transpose_tile(t)
                if gi == 0 and 1 <= t < N_CTRL:
                    bc_expert(t)
                # xs[:, k, ci, :] = X^T * wb — layout [k, ci, b] keeps every
                # operand's innermost stride 1 so the DVE runs in 2x mode.
                xs = xspool.tile([P, KS, cpg, P], BF16, tag=f"xs{gi}", bufs=2)
                nc.vector.tensor_mul(
                    xs,
                    xts[t][:, :, None, :].to_broadcast([P, KS, cpg, P]),
                    wb[:, None, c0:c0 + cpg, ts(t, P)].to_broadcast(
                        [P, KS, cpg, P]
                    ),
                )
                if gi == 0:
                    accs[t] = ps_pool.tile([P, D_OUT], F32, tag="ps",
                                           name=f"acc{t}")
                    nc.tensor.matmul(
                        accs[t], lhsT=wt_pad[:, ts(t, P)], rhs=b_pad,
                        start=True, stop=False,
                    )
                for ci in range(cpg):
                    c = c0 + ci
                    for k in range(KS):
                        nc.tensor.matmul(
                            accs[t],
                            lhsT=xs[:, k, ci, :],
                            rhs=w_sb[:, c, k, :],
                            start=False,
                            stop=(last_g and ci == cpg - 1 and k == KS - 1),
                        )
                if last_g:
                    o_sb = opool.tile([P, D_OUT], F32, tag="o_sb")
                    if t == N_TILES - 1:
                        # Final tile: split eviction + store into halves so
                        # the out-DMA of the first half overlaps the second
                        # half's eviction (shrinks the kernel tail).
                        for h in range(2):
                            nc.scalar.copy(o_sb[:, ts(h, D_OUT // 2)],
                                           accs[t][:, ts(h, D_OUT // 2)])
                            nc.sync.dma_start(
                                o_d[ts(t, P), ts(h, D_OUT // 2)],
                                o_sb[:, ts(h, D_OUT // 2)],
                            )
                    else:
                        nc.scalar.copy(o_sb, accs[t])
                        nc.sync.dma_start(o_d[ts(t, P), :], o_sb)
        return o_sb


def _split_multi_waits(bir: dict) -> dict:
    """The walrus build in this container supports at most ONE sync-wait per
    instruction ("Too many sync wait commands" at codegen otherwise).  Tile's
    scheduler freely attaches several.  Split: keep the last wait on the
    instruction and hoist the others onto standalone same-engine
    EventSemaphore instructions inserted immediately before it — identical
    semantics (the engine blocks at the same program point)."""
    ctr = 0
    for func in bir["functions"]:
        for bb in func["blocks"]:
            new_insts = []
            for inst in bb["instructions"]:
                si = inst.get("sync_info")
                waits = si.get("on_wait") if si else None
                if waits and len(waits) > 1:
                    for w in waits[:-1]:
                        ctr += 1
                        new_insts.append(
                            {
                                "debug": inst.get("debug", 0),
                                "engine": inst["engine"],
                                "ins": [],
                                "outs": [],
                                "name": f"{inst['name']}-wsplit{ctr}",
                                "opcode": "EventSemaphore",
                                "sync_info": {"on_update": [], "on_wait": [w]},
                            }
                        )
                    si["on_wait"] = [waits[-1]]
                new_insts.append(inst)
            bb["instructions"] = new_insts
    return bir


_ENGINE_SEM_PREFIXES = ("PE_", "DVE_", "Activation_", "SP_", "Pool_")


def _strip_redundant_updates(bir: dict) -> dict:
    """Tile attaches a sem update to EVERY engine instruction, but sem-inc
    register writes serialize (~26 ns each) — ~8.5 us of pure PE tail here.
    Engine instructions complete strictly in program order, so an update is
    only needed where some wait actually tests that value.  Keep exactly the
    updates whose cumulative count is waited on (plus the final one per sem,
    for the end-of-program barrier) and renumber all wait thresholds.
    DMA sems (DMAHW*/DMASW*) complete out of program order — untouched."""
    for func in bir["functions"]:
        insts = [i for bb in func["blocks"] for i in bb["instructions"]]
        updates = {}   # sem id -> list of update dicts in program order
        waited = {}    # sem id -> set of waited values
        names = {}     # sem id -> ant_name
        odd_modes = set()   # sems with non-standard update/wait modes
        for inst in insts:
            si = inst.get("sync_info")
            if not si:
                continue
            for u in si.get("on_update") or []:
                names[u["id"]] = u.get("ant_name", "")
                updates.setdefault(u["id"], []).append(u)
                if u.get("update_mode") != "sem-inc":
                    odd_modes.add(u["id"])
            for w in si.get("on_wait") or []:
                names[w["id"]] = w.get("ant_name", "")
                waited.setdefault(w["id"], set()).add(w["wait_value"])
                if w.get("wait_mode") != "sem-ge-imm":
                    odd_modes.add(w["id"])
        remap = {}     # sem id -> {old value -> new value}
        drop = set()   # ids of update dicts to remove
        for sem, ups in updates.items():
            name = names.get(sem, "")
            if not name.startswith(_ENGINE_SEM_PREFIXES):
                continue
            if sem in odd_modes:
                continue
            if any(u.get("update_value") not in (None, 1) for u in ups):
                continue
            if any(not (1 <= v <= len(ups)) for v in waited.get(sem, ())):
                continue
            keep_idx = {len(ups) - 1}
            for v in waited.get(sem, ()):  # v-th update makes count reach v
                if 1 <= v <= len(ups):
                    keep_idx.add(v - 1)
            vmap = {}
            new_count = 0
            for i, u in enumerate(ups):
                if i in keep_idx:
                    new_count += 1
                else:
                    drop.add(id(u))
                vmap[i + 1] = new_count
            remap[sem] = vmap
        for inst in insts:
            si = inst.get("sync_info")
            if not si:
                continue
            ups = si.get("on_update")
            if ups:
                si["on_update"] = [u for u in ups if id(u) not in drop]
            for w in si.get("on_wait") or []:
                vmap = remap.get(w["id"])
                if vmap and w["wait_value"] in vmap:
                    w["wait_value"] = vmap[w["wait_value"]]
    return bir


_ORIG_TO_JSON_BYTES = bass.Bass.to_json_bytes


def _patched_to_json_bytes(self) -> bytes:
    bir = json.loads(_ORIG_TO_JSON_BYTES(self))
    _strip_redundant_updates(bir)
    _split_multi_waits(bir)
    return json.dumps(bir).encode()


_NC_CACHE = {}


def _build(reps: int = 1, serial: bool = False) -> bass.Bass:
    key = (reps, serial)
    if key in _NC_CACHE:
        return _NC_CACHE[key]
    nc = bass.Bass(
        "TRN2",
        target_bir_lowering=False,
        debug=False,
        enable_asserts=False,
        num_devices=N_CORES,
    )
    x_d = nc.dram_tensor("x_in", [B_SHARD, D_IN], F32, kind="ExternalInput").ap()
    wt_d = nc.dram_tensor("wt_in", [B_SHARD, N_CTRL], F32, kind="ExternalInput").ap()
    w_d = nc.dram_tensor("w_in", [N_CTRL, D_IN, D_OUT], F32, kind="ExternalInput").ap()
    b_d = nc.dram_tensor("b_in", [N_CTRL, D_OUT], F32, kind="ExternalInput").ap()
    o_d = nc.dram_tensor("out", [B_SHARD, D_OUT], F32, kind="ExternalOutput").ap()
    with tile.TileContext(nc) as tc:
        with tc.tile_pool(name="global_const", bufs=1) as gconst:
            identity_bf = _consts(nc, gconst)
            gate = None
            for _ in range(reps):
                out_tile = _body(
                    nc, tc, x_d, wt_d, w_d, b_d, o_d, identity_bf,
                    gate=gate,
                )
                if serial:
                    gate = out_tile
    nc.to_json_bytes = types.MethodType(_patched_to_json_bytes, nc)
    _NC_CACHE[key] = nc
    return nc


def kernel(inputs, weights, w, b, _trace=False, _reps=1, _serial=False):
    nc = _build(_reps, _serial)
    inputs = np.ascontiguousarray(inputs, dtype=np.float32)
    weights = np.ascontiguousarray(weights, dtype=np.float32)
    w = np.ascontiguousarray(w, dtype=np.float32)
    b = np.ascontiguousarray(b, dtype=np.float32)

    in_maps = []
    for i in range(N_CORES):
        sl = slice(i * B_SHARD, (i + 1) * B_SHARD)
        in_maps.append(
            {
                "x_in": inputs[sl],
                "wt_in": weights[sl],
                "w_in": w,
                "b_in": b,
            }
        )
    res = run_bass_kernel_spmd(
        nc, in_maps, core_ids=list(range(N_CORES)), trace=_trace
    )
    out = np.concatenate([r["out"] for r in res.results], axis=0)
    if _trace:
        return out, res
    return out
